# revision 19
# baseline (speedup 1.0000x reference)
"""Trainium2 Bass kernel for nn_EuclideanCodebook (VQ codebook lookup).

For each of 65536 tokens (d=128), finds argmax_k of
    dist[t,k] = -(||x_t||^2 - 2 x_t.e_k + ||e_k||^2)
over K=1024 codes (equivalently argmax of 2 x.e_k - ||e_k||^2), returns
(quantize[...,128] f32, embed_ind[...] int32).

Sharding: data-parallel over the flattened token dim across 8 cores;
codebook replicated (inlined into the NEFF as constants).

Device pipeline (per core: 64 tiles of 128 tokens):
  - 16-tile slab DMA-transpose loads of fp16 hi/lo splits of x (exact
    fp32 emulation: x.e = xh.eh + xh.el + xl.eh, max err ~2e-5 on HW)
  - PE: per tile 8 matmuls (6 score + 2 contract-2 bias rows) into a
    two-bank [128,1024] PSUM tile, stationary-major order so each of
    the 3 weight loads (xh, xl, ones) is reused across both halves
  - ACT: PSUM -> SBUF copy -> S [128,1024] exact biased scores
  - DVE: max8 -> row max; scalar_tensor_tensor (S==mx)*iotaRev with
    sum-accumulator -> reversed argmax
  - ACT: idx = relu(1023 - idxrev) (ties clamp to 0, ~never happens)
  - gpsimd indirect DMA per tile: gather embed rows -> slab staging;
    per-slab strided DMA writes quantize rows
  - index staging -> PE transpose -> int32 [64,128] -> DRAM
"""
import os
import numpy as np

import concourse.bacc as bacc
import concourse.bass as bass
import concourse.mybir as mybir
from concourse.tile import TileContext
from concourse.bass_utils import run_bass_kernel_spmd
from concourse.masks import make_identity

N_CORES = 8
DIM = 128
K = 1024
TOKENS = 16 * 4096
TOK_PER_CORE = TOKENS // N_CORES      # 8192
P = 128
N_TILES = TOK_PER_CORE // P           # 64
SLAB = 16                             # tiles per DMA slab
N_SLABS = N_TILES // SLAB
GROUP = 3                             # tiles whose PSUM chains interleave

LAST_RESULTS = None  # test harness reads .exec_time_ns when BASS_TRACE=1


def _build_program(consts, reps=1, stage='full'):
    f16 = mybir.dt.float16
    f32 = mybir.dt.float32
    nc = bacc.Bacc(trn_type="TRN2", target_bir_lowering=False, debug=False,
                   num_devices=N_CORES)

    xh_d = nc.declare_dram_parameter("xh", [TOK_PER_CORE, DIM], f16, isOutput=False)
    xl_d = nc.declare_dram_parameter("xl", [TOK_PER_CORE, DIM], f16, isOutput=False)
    quant_d = nc.declare_dram_parameter("quant", [TOK_PER_CORE, DIM], f32,
                                        isOutput=True)
    ind_d = nc.declare_dram_parameter("ind", [N_TILES, P], mybir.dt.int32,
                                      isOutput=True)

    e2h_d = nc.inline_tensor(consts["e2h"], name="e2h")        # [128, 1024] f16
    e2l_d = nc.inline_tensor(consts["e2l"], name="e2l")        # [128, 1024] f16
    bias_d = nc.inline_tensor(consts["bias2"], name="bias2")   # [2, 1024] f16
    ones_d = nc.inline_tensor(consts["ones2"], name="ones2")   # [2, 128] f16
    iot_d = nc.inline_tensor(consts["iotarev"], name="iotarev")  # [128,1024] f32
    emb_d = nc.inline_tensor(consts["embed"], name="embedtbl")   # [1024, 128] f32

    with TileContext(nc) as tc:
        with (
            tc.tile_pool(name="const", bufs=1) as cpool,
            tc.tile_pool(name="slab", bufs=2) as slpool,
            tc.tile_pool(name="scores", bufs=4) as spool,
            tc.tile_pool(name="small", bufs=8) as mpool,
            tc.tile_pool(name="stage", bufs=1) as stpool,
            tc.tile_pool(name="qslab", bufs=2) as qpool,
            tc.tile_pool(name="psum", bufs=3, space="PSUM") as pspool,
            tc.tile_pool(name="psidx", bufs=1, space="PSUM") as psidx,
        ):
            # --- constants to SBUF ---
            e2h = cpool.tile([DIM, K], f16, tag="e2h")
            e2l = cpool.tile([DIM, K], f16, tag="e2l")
            bias2 = cpool.tile([2, K], f16, tag="bias2")
            ones2 = cpool.tile([2, P], f16, tag="ones2")
            iotarev = cpool.tile([P, K], f32, tag="iotarev")
            ident = cpool.tile([P, P], f32, tag="ident")
            c1023 = cpool.tile([P, 1], f32, tag="c1023")
            nc.sync.dma_start(e2h[:], e2h_d[:])
            nc.sync.dma_start(e2l[:], e2l_d[:])
            nc.sync.dma_start(bias2[:], bias_d[:])
            nc.sync.dma_start(ones2[:], ones_d[:])
            nc.sync.dma_start(iotarev[:], iot_d[:])
            make_identity(nc, ident[:])
            nc.vector.memset(c1023[:], 1023.0)

            idxstage = stpool.tile([P, N_TILES], f32, tag="idxstage")

            for rep in range(reps):
                slabs = {}       # s -> (xhT, xlT)
                qslabs = {}      # s -> staging tile

                def ensure_slab(s):
                    if s not in slabs:
                        srows = slice(s * SLAB * P, (s + 1) * SLAB * P)
                        xhT = slpool.tile([DIM, SLAB * P], f16, tag="xhT")
                        xlT = slpool.tile([DIM, SLAB * P], f16, tag="xlT")
                        nc.sync.dma_start_transpose(xhT[:], xh_d[srows, :])
                        nc.sync.dma_start_transpose(xlT[:], xl_d[srows, :])
                        slabs[s] = (xhT, xlT)
                        qslabs[s] = qpool.tile([P, SLAB * DIM], f32, tag="qs",
                                               name=f"qs_{s}")
                    return slabs[s], qslabs[s]

                def flush_qslab(s):
                    srows = slice(s * SLAB * P, (s + 1) * SLAB * P)
                    nc.scalar.dma_start(
                        out=quant_d[srows, :].rearrange("(c p) d -> p c d", p=P),
                        in_=qslabs[s][:].rearrange("p (c d) -> p c d", d=DIM),
                    )

                for g in range(0, N_TILES, GROUP):
                    tiles = [t for t in range(g, min(g + GROUP, N_TILES))]
                    ops = {}
                    for t in tiles:
                        (xhT, xlT), _ = ensure_slab(t // SLAB)
                        tsl = slice((t % SLAB) * P, (t % SLAB + 1) * P)
                        ops[t] = (xhT[:, tsl], xlT[:, tsl],
                                  pspool.tile([P, K], f32, tag="ph",
                                              name=f"ph_{t}"))
                        # stationary-major order: reuse loaded weights across
                        # both PSUM halves; 3 weight loads per tile
                        xh_t, xl_t, ph = ops[t]
                        for m, (lhs, rhsm) in enumerate([
                            (xh_t, e2h), (xh_t, e2l), (xl_t, e2h),
                            (ones2[:], bias2),
                        ]):
                            for h in range(2):
                                sl = slice(h * 512, (h + 1) * 512)
                                nc.tensor.matmul(
                                    ph[:, sl], lhs, rhsm[:, sl],
                                    start=(m == 0), stop=(m == 3))
                    if stage == "mm":
                        for t in tiles:
                            nc.vector.tensor_copy(
                                idxstage[:, t:t + 1], ops[t][2][:, 0:1])
                        continue
                    for t in tiles:
                        xh_t, xl_t, ph = ops[t]
                        S = spool.tile([P, K], f32, tag="S")
                        nc.scalar.copy(S[:], ph[:])

                        if stage == "evac":
                            nc.vector.tensor_copy(idxstage[:, t:t + 1], S[:, 0:1])
                            continue
                        m8 = mpool.tile([P, 8], f32, tag="m8")
                        nc.vector.max(out=m8[:], in_=S[:])
                        if stage == "max":
                            nc.vector.tensor_copy(idxstage[:, t:t + 1], m8[:, 0:1])
                            continue

                        scrap = spool.tile([P, K], f32, tag="scrap")
                        idxrev = mpool.tile([P, 1], f32, tag="idxrev")
                        nc.vector.scalar_tensor_tensor(
                            out=scrap[:], in0=S[:], scalar=m8[:, 0:1],
                            in1=iotarev[:],
                            op0=mybir.AluOpType.is_equal,
                            op1=mybir.AluOpType.mult,
                            accum_out=idxrev[:],
                        )

                        # idx = relu(1023 - idxrev); ties sum -> clamp to 0
                        idxf = mpool.tile([P, 1], f32, tag="idxf")
                        nc.scalar.activation(
                            idxf[:], idxrev[:],
                            mybir.ActivationFunctionType.Relu,
                            bias=c1023[:, 0:1], scale=-1.0,
                        )
                        nc.vector.tensor_copy(idxstage[:, t:t + 1], idxf[:])
                        idxu = mpool.tile([P, 1], mybir.dt.uint32, tag="idxu")
                        nc.vector.tensor_copy(idxu[:], idxf[:])

                        if stage == "stt":
                            continue
                        s = t // SLAB
                        i = t % SLAB
                        nc.gpsimd.indirect_dma_start(
                            out=qslabs[s][:, i * DIM:(i + 1) * DIM],
                            out_offset=None,
                            in_=emb_d[:],
                            in_offset=bass.IndirectOffsetOnAxis(
                                ap=idxu[:, 0:1], axis=0),
                        )
                        if i == SLAB - 1 and stage == "full":
                            flush_qslab(s)

                pt = psidx.tile([N_TILES, P], f32, tag="pt")
                nc.tensor.transpose(pt[:], idxstage[:], ident[:])
                idxi = stpool.tile([N_TILES, P], mybir.dt.int32, tag="idxi")
                nc.vector.tensor_copy(idxi[:], pt[:])
                nc.sync.dma_start(ind_d[:], idxi[:])

    nc.compile()
    return nc


def kernel(x: np.ndarray, embed: np.ndarray):
    global LAST_RESULTS
    x = np.asarray(x)
    embed = np.asarray(embed)
    shape = x.shape
    flat = np.ascontiguousarray(x.reshape(-1, shape[-1]).astype(np.float32,
                                                               copy=False))
    embed = np.ascontiguousarray(embed.astype(np.float32, copy=False))

    # host-side prep: fp16 hi/lo splits and codebook transforms
    xh = flat.astype(np.float16)
    xl = (flat - xh.astype(np.float32)).astype(np.float16)

    e2t = np.ascontiguousarray((2.0 * embed.astype(np.float32)).T)  # [128,1024]
    e2h = e2t.astype(np.float16)
    e2l = (e2t - e2h.astype(np.float32)).astype(np.float16)
    e_sq = np.sum(embed * embed, axis=1, dtype=np.float32)          # [1024]
    bias = -e_sq
    bias_h = bias.astype(np.float16)
    bias_l = (bias - bias_h.astype(np.float32)).astype(np.float16)
    bias2 = np.stack([bias_h, bias_l], axis=0)                      # [2, 1024]
    ones2 = np.ones((2, P), dtype=np.float16)
    iotarev = np.broadcast_to(
        (1023.0 - np.arange(K, dtype=np.float32))[None, :], (P, K)
    ).copy()

    consts = {
        "e2h": e2h, "e2l": e2l, "bias2": bias2, "ones2": ones2,
        "iotarev": iotarev, "embed": embed,
    }
    nc = _build_program(consts)

    in_maps = []
    for c in range(N_CORES):
        rows = slice(c * TOK_PER_CORE, (c + 1) * TOK_PER_CORE)
        in_maps.append({
            "xh": np.ascontiguousarray(xh[rows]),
            "xl": np.ascontiguousarray(xl[rows]),
        })

    res = run_bass_kernel_spmd(
        nc, in_maps, list(range(N_CORES)),
        trace=bool(os.environ.get("BASS_TRACE")),
    )
    LAST_RESULTS = res

    quant = np.concatenate([res.results[c]["quant"] for c in range(N_CORES)],
                           axis=0)
    ind = np.concatenate(
        [res.results[c]["ind"].reshape(-1) for c in range(N_CORES)], axis=0
    ).astype(np.int32)

    quantize = quant.reshape(shape)
    embed_ind = ind.reshape(shape[:-1])
    return quantize, embed_ind
